# revision 34
# baseline (speedup 1.0000x reference)
"""HarsanyiNet forward on 8 TRN2 NeuronCores (Bass/Tile), fused single launch.

Model (reference):
    harsanyi_block(x, v, fc):
        m = (v > 0)                                    # [O, I] mask
        delta = prod_i [ tanh(g*|x_i|) if m else 1 ]   # [B, O]
        h = relu((x @ (fc*m).T) * delta)
    y = h0 @ head0.T + h1 @ head1.T   (two blocks, h0 feeds block 1)

Key structure:
  * delta in log space: delta = exp(L @ m.T), L = log(tanh(g*|x|)) =
    ln(1-z) - ln(1+z) with z = exp(-2g|x|) -> only {exp, ln} on ScalarE,
    one activation-table set, loaded once (warm op at kernel start).
  * ONE launch for both layers.  A 2-launch version pays the fixed
    per-launch cost twice (~6us preamble + ~9us semaphore teardown);
    fusing pays it once.  Layer 0 is computed IN FULL on every core
    (weights replicated) so layer 1 needs no cross-core gather of h0
    (collectives are unavailable in this execution environment); layer
    1 is sharded across cores by output-hidden chunk (128 rows/core).
  * Mask m and w = fc*m are folded on the host (static weight
    transforms); all x-dependent arithmetic runs on device.  Masks ride
    as fp8e4 (exact for 0/1) stationary operands against bf16 moving
    operands -- halves the dominant m0 DMA traffic.
  * DMA: ~0.7us issue cost per dma_start and ~27GB/s per ring at wide
    rows -> few, wide transfers.  m1/w1 are packed into the same DRAM
    tensors as m0/w0; 7 input calls total, split across the gpsimd
    (SWDGE) and sync (HWDGE) issue engines which get 8 rings each.
  * PE order: ~60 tiny warm-up matmuls into a scratch PSUM bank (keeps
    the PE p-state ramped through the DMA window), then the HL0 sweep
    (needs only weights + x), then the S0 sweep (needs the L0 chain),
    then layer-1/head matmuls.  S0/HL0 accumulate into per-PAIR psum
    tiles so the trailing delta0/h0/L1 chains unblock pair-by-pair
    instead of waiting for the whole sweep (whole-tile dependency).
  * The scalar queue runs each pair's exp(S0) one step AHEAD of the
    z1/p1/q1 block so the in-order queue pipelines across pairs
    instead of serializing each pair's cross-engine ping-pong.
  * SPMD trick: each core's weight blocks are ROTATED so chunk slot k
    holds layer-0 output-chunk (core+k)%8; layer-1 contraction slots
    rotate identically.  One program for all cores; slot 0 is always
    "this core's own" head0 chunk.
  * Only output: y_part [C, B] f32 per core; host sums the partials.
"""
import sys

import numpy as np

sys.path.insert(0, "/opt/trn_rl_repo")

import ml_dtypes  # noqa: E402

from concourse import bacc, mybir, tile  # noqa: E402
from concourse.alu_op_type import AluOpType  # noqa: E402
from concourse.bass_utils import run_bass_kernel_spmd  # noqa: E402
from concourse.tile_rust import add_dep_helper  # noqa: E402


def _order(after, before, why):
    """Order-only scheduling edge: `after` runs after `before`."""
    add_dep_helper(getattr(after, "ins", after), getattr(before, "ins", before),
                   sync=False, reason=why)

B, NIN, HID, C = 64, 1024, 1024, 10
GAMMA = 100.0
N_CORES = 8
KCH = 8                     # 128-row chunks per 1024-long dim
OSH = 128                   # layer-1 output rows per core
KB = KCH * B                # activation columns, chunk-major (512)
NP2 = KCH // 2              # chunk pairs (4)
# mA = [m1 | m0 slots 0..3], mB = m0 slots 4..7 (wA/wB likewise for w)
KA = (1 + 4) * 1024         # 5120 cols
KB2 = 4 * 1024              # 4096 cols
N_WARM = 60                 # PE warm-up matmuls (~60ns each)
# Upper clamp applied to ln(1-z): keeps L finite if the Exp LUT rounds z
# up to 1 (the reference's exact-zero delta becomes a ~e-30000 factor).
LCLAMP = -30000.0
F32 = mybir.dt.float32
BF16 = mybir.dt.bfloat16
FP8 = mybir.dt.float8e4
BF16_NP = ml_dtypes.bfloat16
FP8_NP = ml_dtypes.float8_e4m3
M_FP8 = True                # masks as fp8e4 stationary operands
M_DT = FP8 if M_FP8 else BF16
M_NP = FP8_NP if M_FP8 else BF16_NP

PROFILE = {"enable": False, "trace_kwargs": {}, "runs": []}
_CACHE = {}


def _shrink_kernel_sem_range(start=208):
    """The NEFF epilogue sweeps/restores every semaphore in the kernel
    range (256 - walrus_max) on every engine, ~70ns per sem-op -- ~7us
    of fixed teardown per launch.  This kernel needs far fewer sems, so
    shrink the pool; bass raises at build time if it ever runs out."""
    import concourse.bass as bass_mod

    bass_mod.get_kernel_semaphore_range = lambda: range(start, 256)


def _force_act_table_set(target="natural_log_exp_and_others"):
    """Make the act-table-load pass place every activation in `target`
    (it otherwise picks the first set per function, costing one ~2.7us
    table switch per Exp->Ln transition).  Indices of the table list are
    act_func_set_ids, so ordering is preserved and other sets are
    emptied."""
    import concourse.bacc as bacc_mod
    from concourse.hw_specs import get_activation_tables as real_tabs

    def patched(arch):
        tabs = real_tabs(arch)
        return {name: (funcs if name == target else set())
                for name, funcs in tabs.items()}

    bacc_mod.get_activation_tables = patched


def _build():
    _shrink_kernel_sem_range()
    _force_act_table_set()
    nc = bacc.Bacc("TRN2", target_bir_lowering=False, debug=False,
                   num_devices=N_CORES, enable_asserts=False)
    xT = nc.declare_dram_parameter("xT", [128, KB], F32, isOutput=False)
    mA = nc.declare_dram_parameter("mA", [128, KA], M_DT, isOutput=False)
    mB = nc.declare_dram_parameter("mB", [128, KB2], M_DT, isOutput=False)
    wA = nc.declare_dram_parameter("wA", [128, KA], BF16, isOutput=False)
    wB = nc.declare_dram_parameter("wB", [128, KB2], BF16, isOutput=False)
    hd0T = nc.declare_dram_parameter("hd0T", [128, C], F32, isOutput=False)
    hd1T = nc.declare_dram_parameter("hd1T", [128, C], F32, isOutput=False)
    y_part = nc.declare_dram_parameter("y_part", [C, B], F32, isOutput=True)
    Act = mybir.ActivationFunctionType

    with tile.TileContext(nc) as tc:
        with (
            tc.tile_pool(name="sb", bufs=1) as sb,
            tc.tile_pool(name="ps", bufs=1, space="PSUM") as ps,
        ):
            # ---------------- SBUF tiles
            xf = sb.tile([128, KB], F32)
            xb = sb.tile([128, KB], BF16)
            ma = sb.tile([128, KA], M_DT)
            mb = sb.tile([128, KB2], M_DT)
            wa = sb.tile([128, KA], BF16)
            wb = sb.tile([128, KB2], BF16)
            hd0 = sb.tile([128, C], F32)
            hd1 = sb.tile([128, C], F32)
            eps = sb.tile([128, 1], F32)
            warm_i = sb.tile([128, 1], F32)
            warm_o = sb.tile([128, 1], F32)
            wtiny = sb.tile([128, 32], BF16)
            a0 = sb.tile([128, KB], F32)
            z0 = sb.tile([128, KB], F32)
            p0 = sb.tile([128, KB], F32)
            q0 = sb.tile([128, KB], F32)
            L0f = sb.tile([128, KB], F32)
            L0 = sb.tile([128, KB], BF16)
            d0 = sb.tile([128, KB], F32)
            h0f = sb.tile([128, KB], F32)
            h0b = sb.tile([128, KB], BF16)
            z1 = sb.tile([128, KB], F32)
            p1 = sb.tile([128, KB], F32)
            q1 = sb.tile([128, KB], F32)
            L1f = sb.tile([128, KB], F32)
            L1 = sb.tile([128, KB], BF16)
            d1 = sb.tile([128, B], F32)
            h1f = sb.tile([128, B], F32)
            yo = sb.tile([C, B], F32)
            # ---------------- PSUM: per-PAIR accumulation tiles so the
            # chains unblock as each pair of chunks closes
            # PSUM tiles are bank-granular (2KB/partition, 8 banks) and
            # dependencies are whole-tile, so chain-gating tiles must be
            # pair-granular: 3 S0 pair tiles + 3 HL0 pair tiles + one
            # shared pair-3 tile (S0|HL0 halves; both close last anyway)
            # + one layer-1 bank ([S1|HL1|Y], which also absorbs the PE
            # warm-up writes -- S1's start=True overwrites them) = 8.
            S0P = [ps.tile([128, 2 * B], F32, name=f"S0P{p}")
                   for p in range(NP2 - 1)]
            HL0P = [ps.tile([128, 2 * B], F32, name=f"HL0P{p}")
                    for p in range(NP2 - 1)]
            SH3 = ps.tile([128, 4 * B], F32)

            def s0_ap(p, lo=0, hi=2 * B):        # pair p, col range
                return S0P[p][:, lo:hi] if p < 3 else SH3[:, lo:hi]

            def hl0_ap(p, lo=0, hi=2 * B):
                return (HL0P[p][:, lo:hi] if p < 3
                        else SH3[:, 2 * B + lo:2 * B + hi])
            L1P = ps.tile([128, 3 * B], F32)     # [S1 | HL1 | Y] slices
            S1p = L1P[:, 0:B]
            HL1p = L1P[:, B:2 * B]
            Yp = L1P[:C, 2 * B:2 * B + B]
            WRMp = SH3[:, 0:32]

            # stationary slice for layer-0 chunk (ko, ki); m1/w1 at slot 0
            def msl(ko, ki):
                base = (1 + ko) * 1024 if ko < 4 else (ko - 4) * 1024
                return slice(base + ki * 128, base + (ki + 1) * 128)

            # ---------------- DMA issue (gpsimd = SWDGE, sync = HWDGE;
            # each gets its own 8 rings)
            prev = {"g": None, "s": None}

            def dma(eng, key, dst, src):
                op = eng.dma_start(dst, src)
                if prev[key] is not None:
                    _order(op, prev[key], f"{key}-dma order")
                prev[key] = op
                return op

            # All input DMAs issue from sync (HWDGE): it reaches all 16
            # rings and they spin up early; SWDGE (gpsimd) rings start
            # several us late and only reach ~150GB/s.  Order = arrival
            # order = the PE consumption order below.
            dma(nc.sync, "s", xf[:], xT[:, :])
            dma(nc.sync, "s", hd0[:], hd0T[:, :])
            dma(nc.sync, "s", hd1[:], hd1T[:, :])
            dma(nc.sync, "s", ma[:], mA[:, :])
            dma(nc.sync, "s", wa[:], wA[:, :])
            # the tail pairs arrive as small interleaved m/w pieces so
            # the last chain step depends on as little late data as
            # possible: pair-2 (chunks 4,5) complete, then pair-3
            HB2 = 2 * 1024
            dma(nc.sync, "s", mb[:, :HB2], mB[:, :HB2])
            dma(nc.sync, "s", wb[:, :HB2], wB[:, :HB2])
            dma(nc.sync, "s", mb[:, HB2:], mB[:, HB2:])
            dma(nc.sync, "s", wb[:, HB2:], wB[:, HB2:])

            # ---------------- constants + act-table warm
            nc.vector.memset(eps[:], -1e-6)
            nc.vector.memset(warm_i[:], 0.0)
            nc.vector.memset(wtiny[:], 0.0)
            warm = nc.scalar.activation(warm_o[:], warm_i[:], Act.Exp)

            tail = {"sc": warm, "ve": None, "pe": None}

            def q(key, op):
                if tail[key] is not None:
                    _order(op, tail[key], f"{key} queue order")
                tail[key] = op
                return op

            def mm(out_ap, lhs_ap, rhs_ap, start, stop, skip=False):
                return q("pe", nc.tensor.matmul(out_ap, lhs_ap, rhs_ap,
                                                start=start, stop=stop,
                                                skip_group_check=skip))

            # ---------------- layer-0 L chain (single shot; feeds S0).
            # xb first: the PE warm-ups use it as their stationary.
            q("ve", nc.vector.tensor_copy(xb[:], xf[:]))
            q("ve", nc.vector.scalar_tensor_tensor(
                a0[:], xf[:], -1.0, xf[:],
                op0=AluOpType.mult, op1=AluOpType.max))
            q("sc", nc.scalar.activation(z0[:], a0[:], Act.Exp,
                                         scale=-2.0 * GAMMA, bias=eps[:]))
            q("sc", nc.scalar.activation(p0[:], z0[:], Act.Ln,
                                         bias=1.0, scale=-1.0))
            q("sc", nc.scalar.activation(q0[:], z0[:], Act.Ln,
                                         bias=1.0, scale=1.0))
            q("ve", nc.vector.scalar_tensor_tensor(
                L0f[:], p0[:], LCLAMP, q0[:],
                op0=AluOpType.max, op1=AluOpType.subtract))
            q("ve", nc.vector.tensor_copy(L0[:], L0f[:]))

            # ---------------- PE: warm-ups (keep the p-state ramped
            # through the DMA window), then S0/HL0 half-sweeps in DMA
            # arrival order: S0[0:4] (ma+L0), HL0[0:4] (wa), S0[4:8]
            # (mb), HL0[4:8] (wb).
            for i in range(N_WARM):
                mm(WRMp, xb[:, 0:128], wtiny[:],
                   start=True, stop=True, skip=True)

            def s0_sweep(kos):
                for ko in kos:
                    lo = (ko % 2) * B
                    for ki in range(KCH):
                        mm(s0_ap(ko // 2, lo, lo + B),
                           (ma if ko < 4 else mb)[:, msl(ko, ki)],
                           L0[:, ki * B:(ki + 1) * B],
                           start=(ki == 0), stop=(ki == KCH - 1))

            def hl0_sweep(kos):
                for ko in kos:
                    lo = (ko % 2) * B
                    for ki in range(KCH):
                        mm(hl0_ap(ko // 2, lo, lo + B),
                           (wa if ko < 4 else wb)[:, msl(ko, ki)],
                           xb[:, ki * B:(ki + 1) * B],
                           start=(ki == 0), stop=(ki == KCH - 1))

            s0_sweep(range(0, 4))
            hl0_sweep(range(0, 4))
            s0_sweep(range(4, 6))
            hl0_sweep(range(4, 6))
            s0_sweep(range(6, 8))
            hl0_sweep(range(6, 8))

            # ---------------- trailing chains, one pair per step, with
            # d0 running one step ahead on the scalar queue
            def bsl(p):
                return slice(2 * p * B, (2 * p + 2) * B)

            def d0_step(p):
                q("sc", nc.scalar.activation(d0[:, bsl(p)], s0_ap(p),
                                             Act.Exp))

            def v_h0(p):
                bs = bsl(p)
                q("ve", nc.vector.scalar_tensor_tensor(
                    h0f[:, bs], hl0_ap(p), 0.0, d0[:, bs],
                    op0=AluOpType.max, op1=AluOpType.mult))
                q("ve", nc.vector.tensor_copy(h0b[:, bs], h0f[:, bs]))

            def sc_l1(p):
                bs = bsl(p)
                q("sc", nc.scalar.activation(z1[:, bs], h0f[:, bs], Act.Exp,
                                             scale=-2.0 * GAMMA, bias=eps[:]))
                q("sc", nc.scalar.activation(p1[:, bs], z1[:, bs], Act.Ln,
                                             bias=1.0, scale=-1.0))
                q("sc", nc.scalar.activation(q1[:, bs], z1[:, bs], Act.Ln,
                                             bias=1.0, scale=1.0))

            def v_l1(p):
                bs = bsl(p)
                q("ve", nc.vector.scalar_tensor_tensor(
                    L1f[:, bs], p1[:, bs], LCLAMP, q1[:, bs],
                    op0=AluOpType.max, op1=AluOpType.subtract))
                q("ve", nc.vector.tensor_copy(L1[:, bs], L1f[:, bs]))

            d0_step(0)
            v_h0(0)
            for p in range(NP2):
                if p + 1 < NP2:
                    d0_step(p + 1)
                    v_h0(p + 1)
                sc_l1(p)
                v_l1(p)

            # ---------------- layer-1 + head matmuls, then finale
            for j in range(KCH):
                lsl = slice(j * 128, (j + 1) * 128)
                bsj = slice(j * B, (j + 1) * B)
                mm(S1p, ma[:, lsl], L1[:, bsj],
                   start=(j == 0), stop=(j == KCH - 1), skip=True)
                mm(HL1p, wa[:, lsl], h0b[:, bsj],
                   start=(j == 0), stop=(j == KCH - 1), skip=True)
            # head0 partial for this core's own chunk (slot 0)
            mm(Yp, hd0[:, :], h0f[:, 0:B], start=True, stop=False, skip=True)
            q("sc", nc.scalar.activation(d1[:], S1p, Act.Exp))
            q("ve", nc.vector.scalar_tensor_tensor(
                h1f[:], HL1p, 0.0, d1[:],
                op0=AluOpType.max, op1=AluOpType.mult))
            mm(Yp, hd1[:, :], h1f[:], start=False, stop=True, skip=True)
            q("ve", nc.vector.tensor_copy(yo[:], Yp))
            dma(nc.sync, "s", y_part[:, :], yo[:])
    nc.compile()
    return nc


def _prep(x, v0, fc0, head0, v1, fc1, head1):
    """Host-side weight preprocessing -> per-core in_maps.

    Per core c, layer-0 output-chunk slot k holds chunk (c+k)%8 and
    layer-1 contraction slot k rotates identically, so one SPMD program
    serves all cores.  m1/w1 occupy slot 0 of the mA/wA packs."""
    m0 = (np.asarray(v0) > 0).astype(np.float32)
    w0 = np.asarray(fc0, np.float32) * m0
    m1 = (np.asarray(v1) > 0).astype(np.float32)
    w1 = np.asarray(fc1, np.float32) * m1
    xT = np.asarray(x, np.float32).T                      # [1024, 64]
    xc = np.ascontiguousarray(
        xT.reshape(KCH, 128, B).transpose(1, 0, 2).reshape(128, KB))
    # [ki, ip, oc, op] blocks of the transposed layer-0 weights
    m0blk = m0.T.reshape(KCH, 128, KCH, 128)
    w0blk = w0.T.reshape(KCH, 128, KCH, 128)
    head0 = np.asarray(head0, np.float32)
    head1 = np.asarray(head1, np.float32)
    in_maps = []
    for c in range(N_CORES):
        perm = [(c + k) % KCH for k in range(KCH)]
        # slot-major k, then ki, then op: [ip, k, ki, op] -> [128, 8192]
        m0c = m0blk[:, :, perm, :].transpose(1, 2, 0, 3).reshape(128, -1)
        w0c = w0blk[:, :, perm, :].transpose(1, 2, 0, 3).reshape(128, -1)
        sl = slice(c * OSH, (c + 1) * OSH)
        m1t = m1[sl].T.reshape(KCH, 128, OSH)             # [ic, ip, o]
        w1t = w1[sl].T.reshape(KCH, 128, OSH)
        m1c = m1t[perm].transpose(1, 0, 2).reshape(128, KCH * OSH)
        w1c = w1t[perm].transpose(1, 0, 2).reshape(128, KCH * OSH)
        in_maps.append({
            "xT": xc,
            "mA": np.ascontiguousarray(
                np.concatenate([m1c, m0c[:, :4 * 1024]], axis=1)).astype(M_NP),
            "mB": np.ascontiguousarray(m0c[:, 4 * 1024:]).astype(M_NP),
            "wA": np.ascontiguousarray(
                np.concatenate([w1c, w0c[:, :4 * 1024]], axis=1)
            ).astype(BF16_NP),
            "wB": np.ascontiguousarray(w0c[:, 4 * 1024:]).astype(BF16_NP),
            "hd0T": np.ascontiguousarray(head0[:, sl].T),
            "hd1T": np.ascontiguousarray(head1[:, sl].T),
        })
    return in_maps


def kernel(x, v0, fc0, head0, v1, fc1, head1):
    nc = _CACHE.get("nc")
    if nc is None:
        nc = _CACHE["nc"] = _build()
    in_maps = _prep(x, v0, fc0, head0, v1, fc1, head1)
    kwargs = {}
    if PROFILE["enable"]:
        kwargs = {"trace": True, **PROFILE["trace_kwargs"]}
    res = run_bass_kernel_spmd(nc, in_maps, core_ids=list(range(N_CORES)),
                               **kwargs)
    if PROFILE["enable"]:
        PROFILE["runs"].append(res)
    y = np.zeros((C, B), np.float32)
    for c in range(N_CORES):
        y += res.results[c]["y_part"]
    return np.ascontiguousarray(y.T).astype(np.float32)


# revision 36
# speedup vs baseline: 1.0483x; 1.0483x over previous
"""HarsanyiNet forward on 8 TRN2 NeuronCores (Bass/Tile), fused single launch.

Model (reference):
    harsanyi_block(x, v, fc):
        m = (v > 0)                                    # [O, I] mask
        delta = prod_i [ tanh(g*|x_i|) if m else 1 ]   # [B, O]
        h = relu((x @ (fc*m).T) * delta)
    y = h0 @ head0.T + h1 @ head1.T   (two blocks, h0 feeds block 1)

Key structure:
  * delta in log space: delta = exp(L @ m.T), L = log(tanh(g*|x|)) =
    ln(1-z) - ln(1+z) with z = exp(-2g|x|) -> only {exp, ln} on ScalarE,
    one activation-table set, loaded once (warm op at kernel start).
  * ONE launch for both layers.  A 2-launch version pays the fixed
    per-launch cost twice (~6us preamble + ~9us semaphore teardown);
    fusing pays it once.  Layer 0 is computed IN FULL on every core
    (weights replicated) so layer 1 needs no cross-core gather of h0
    (collectives are unavailable in this execution environment); layer
    1 is sharded across cores by output-hidden chunk (128 rows/core).
  * Mask m and w = fc*m are folded on the host (static weight
    transforms); all x-dependent arithmetic runs on device.  Masks ride
    as fp8e4 (exact for 0/1) stationary operands against bf16 moving
    operands -- halves the dominant m0 DMA traffic.
  * DMA: ~0.7us issue cost per dma_start and ~27GB/s per ring at wide
    rows -> few, wide transfers.  m1/w1 are packed into the same DRAM
    tensors as m0/w0; 7 input calls total, split across the gpsimd
    (SWDGE) and sync (HWDGE) issue engines which get 8 rings each.
  * PE order: ~60 tiny warm-up matmuls into a scratch PSUM bank (keeps
    the PE p-state ramped through the DMA window), then the HL0 sweep
    (needs only weights + x), then the S0 sweep (needs the L0 chain),
    then layer-1/head matmuls.  S0/HL0 accumulate into per-PAIR psum
    tiles so the trailing delta0/h0/L1 chains unblock pair-by-pair
    instead of waiting for the whole sweep (whole-tile dependency).
  * The scalar queue runs each pair's exp(S0) one step AHEAD of the
    z1/p1/q1 block so the in-order queue pipelines across pairs
    instead of serializing each pair's cross-engine ping-pong.
  * SPMD trick: each core's weight blocks are ROTATED so chunk slot k
    holds layer-0 output-chunk (core+k)%8; layer-1 contraction slots
    rotate identically.  One program for all cores; slot 0 is always
    "this core's own" head0 chunk.
  * Only output: y_part [C, B] f32 per core; host sums the partials.
"""
import sys

import numpy as np

sys.path.insert(0, "/opt/trn_rl_repo")

import ml_dtypes  # noqa: E402

from concourse import bacc, mybir, tile  # noqa: E402
from concourse.alu_op_type import AluOpType  # noqa: E402
from concourse.bass_utils import run_bass_kernel_spmd  # noqa: E402
from concourse.tile_rust import add_dep_helper  # noqa: E402


def _order(after, before, why):
    """Order-only scheduling edge: `after` runs after `before`."""
    add_dep_helper(getattr(after, "ins", after), getattr(before, "ins", before),
                   sync=False, reason=why)

B, NIN, HID, C = 64, 1024, 1024, 10
GAMMA = 100.0
N_CORES = 8
KCH = 8                     # 128-row chunks per 1024-long dim
OSH = 128                   # layer-1 output rows per core
KB = KCH * B                # activation columns, chunk-major (512)
NP2 = KCH // 2              # chunk pairs (4)
# mA = [m1 | m0 slots 0..3], mB = m0 slots 4..7 (wA/wB likewise for w)
KA = (1 + 4) * 1024         # 5120 cols
KB2 = 4 * 1024              # 4096 cols
N_WARM = 60                 # PE warm-up matmuls (~60ns each)
# Upper clamp applied to ln(1-z): keeps L finite if the Exp LUT rounds z
# up to 1 (the reference's exact-zero delta becomes a ~e-30000 factor).
LCLAMP = -30000.0
F32 = mybir.dt.float32
BF16 = mybir.dt.bfloat16
FP8 = mybir.dt.float8e4
BF16_NP = ml_dtypes.bfloat16
FP8_NP = ml_dtypes.float8_e4m3
M_FP8 = True                # masks as fp8e4 stationary operands
M_DT = FP8 if M_FP8 else BF16
M_NP = FP8_NP if M_FP8 else BF16_NP

PROFILE = {"enable": False, "trace_kwargs": {}, "runs": []}
_CACHE = {}


def _shrink_kernel_sem_range(start=208):
    """The NEFF epilogue sweeps/restores every semaphore in the kernel
    range (256 - walrus_max) on every engine, ~70ns per sem-op -- ~7us
    of fixed teardown per launch.  This kernel needs far fewer sems, so
    shrink the pool; bass raises at build time if it ever runs out."""
    import concourse.bass as bass_mod

    bass_mod.get_kernel_semaphore_range = lambda: range(start, 256)


def _force_act_table_set(target="natural_log_exp_and_others"):
    """Make the act-table-load pass place every activation in `target`
    (it otherwise picks the first set per function, costing one ~2.7us
    table switch per Exp->Ln transition).  Indices of the table list are
    act_func_set_ids, so ordering is preserved and other sets are
    emptied."""
    import concourse.bacc as bacc_mod
    from concourse.hw_specs import get_activation_tables as real_tabs

    def patched(arch):
        tabs = real_tabs(arch)
        return {name: (funcs if name == target else set())
                for name, funcs in tabs.items()}

    bacc_mod.get_activation_tables = patched


def _build():
    _shrink_kernel_sem_range()
    _force_act_table_set()
    nc = bacc.Bacc("TRN2", target_bir_lowering=False, debug=False,
                   num_devices=N_CORES, enable_asserts=False)
    xT = nc.declare_dram_parameter("xT", [128, KB], F32, isOutput=False)
    mA = nc.declare_dram_parameter("mA", [128, KA], M_DT, isOutput=False)
    mB = nc.declare_dram_parameter("mB", [128, KB2], M_DT, isOutput=False)
    wA = nc.declare_dram_parameter("wA", [128, KA], BF16, isOutput=False)
    wB = nc.declare_dram_parameter("wB", [128, KB2], BF16, isOutput=False)
    hd0T = nc.declare_dram_parameter("hd0T", [128, C], F32, isOutput=False)
    hd1T = nc.declare_dram_parameter("hd1T", [128, C], F32, isOutput=False)
    y_part = nc.declare_dram_parameter("y_part", [C, B], F32, isOutput=True)
    Act = mybir.ActivationFunctionType

    with tile.TileContext(nc) as tc:
        with (
            tc.tile_pool(name="sb", bufs=1) as sb,
            tc.tile_pool(name="ps", bufs=1, space="PSUM") as ps,
        ):
            # ---------------- SBUF tiles
            xf = sb.tile([128, KB], F32)
            xb = sb.tile([128, KB], BF16)
            ma = sb.tile([128, KA], M_DT)
            mb = sb.tile([128, KB2], M_DT)
            wa = sb.tile([128, KA], BF16)
            wb = sb.tile([128, KB2], BF16)
            hd0 = sb.tile([128, C], F32)
            hd1 = sb.tile([128, C], F32)
            eps = sb.tile([128, 1], F32)
            warm_i = sb.tile([128, 1], F32)
            warm_o = sb.tile([128, 1], F32)
            wtiny = sb.tile([128, 32], BF16)
            a0 = sb.tile([128, KB], F32)
            z0 = sb.tile([128, KB], F32)
            p0 = sb.tile([128, KB], F32)
            q0 = sb.tile([128, KB], F32)
            L0f = sb.tile([128, KB], F32)
            L0 = sb.tile([128, KB], BF16)
            d0 = sb.tile([128, KB], F32)
            h0f = sb.tile([128, KB], F32)
            h0b = sb.tile([128, KB], BF16)
            z1 = sb.tile([128, KB], F32)
            p1 = sb.tile([128, KB], F32)
            q1 = sb.tile([128, KB], F32)
            L1f = sb.tile([128, KB], F32)
            L1 = sb.tile([128, KB], BF16)
            d1 = sb.tile([128, B], F32)
            h1f = sb.tile([128, B], F32)
            yo = sb.tile([C, B], F32)
            # ---------------- PSUM: per-PAIR accumulation tiles so the
            # chains unblock as each pair of chunks closes
            # PSUM tiles are bank-granular (2KB/partition, 8 banks) and
            # dependencies are whole-tile, so chain-gating tiles must be
            # pair-granular: 3 S0 pair tiles + 3 HL0 pair tiles + one
            # shared pair-3 tile (S0|HL0 halves; both close last anyway)
            # + one layer-1 bank ([S1|HL1|Y], which also absorbs the PE
            # warm-up writes -- S1's start=True overwrites them) = 8.
            S0P = [ps.tile([128, 2 * B], F32, name=f"S0P{p}")
                   for p in range(NP2 - 1)]
            HL0P = [ps.tile([128, 2 * B], F32, name=f"HL0P{p}")
                    for p in range(NP2 - 1)]
            SH3 = ps.tile([128, 4 * B], F32)

            def s0_ap(p, lo=0, hi=2 * B):        # pair p, col range
                return S0P[p][:, lo:hi] if p < 3 else SH3[:, lo:hi]

            def hl0_ap(p, lo=0, hi=2 * B):
                return (HL0P[p][:, lo:hi] if p < 3
                        else SH3[:, 2 * B + lo:2 * B + hi])
            L1P = ps.tile([128, 3 * B], F32)     # [S1 | HL1 | Y] slices
            S1p = L1P[:, 0:B]
            HL1p = L1P[:, B:2 * B]
            Yp = L1P[:C, 2 * B:2 * B + B]
            WRMp = SH3[:, 0:32]

            # stationary slice for layer-0 chunk (ko, ki); m1/w1 at slot 0
            def msl(ko, ki):
                base = (1 + ko) * 1024 if ko < 4 else (ko - 4) * 1024
                return slice(base + ki * 128, base + (ki + 1) * 128)

            # ---------------- DMA issue (gpsimd = SWDGE, sync = HWDGE;
            # each gets its own 8 rings)
            prev = {"g": None, "s": None}

            def dma(eng, key, dst, src):
                op = eng.dma_start(dst, src)
                if prev[key] is not None:
                    _order(op, prev[key], f"{key}-dma order")
                prev[key] = op
                return op

            # All input DMAs issue from sync (HWDGE): it reaches all 16
            # rings and they spin up early; SWDGE (gpsimd) rings start
            # several us late and only reach ~150GB/s.  Order = arrival
            # order = the PE consumption order below.
            dma(nc.sync, "s", xf[:], xT[:, :])
            dma(nc.sync, "s", hd0[:], hd0T[:, :])
            dma(nc.sync, "s", hd1[:], hd1T[:, :])
            dma(nc.sync, "s", ma[:], mA[:, :])
            dma(nc.sync, "s", wa[:], wA[:, :])
            # wb before mb: the last-arriving tensor gates only the
            # exp(S0) hop (d0 <- S0 <- mb), not the whole h0->L1 chain
            dma(nc.sync, "s", wb[:], wB[:, :])
            dma(nc.sync, "s", mb[:], mB[:, :])

            # ---------------- constants + act-table warm
            nc.vector.memset(eps[:], -1e-6)
            nc.vector.memset(warm_i[:], 0.0)
            nc.vector.memset(wtiny[:], 0.0)
            warm = nc.scalar.activation(warm_o[:], warm_i[:], Act.Exp)

            tail = {"sc": warm, "ve": None, "pe": None}

            def q(key, op):
                if tail[key] is not None:
                    _order(op, tail[key], f"{key} queue order")
                tail[key] = op
                return op

            def mm(out_ap, lhs_ap, rhs_ap, start, stop, skip=False):
                return q("pe", nc.tensor.matmul(out_ap, lhs_ap, rhs_ap,
                                                start=start, stop=stop,
                                                skip_group_check=skip))

            # ---------------- layer-0 L chain (single shot; feeds S0).
            # xb first: the PE warm-ups use it as their stationary.
            q("ve", nc.vector.tensor_copy(xb[:], xf[:]))
            q("ve", nc.vector.scalar_tensor_tensor(
                a0[:], xf[:], -1.0, xf[:],
                op0=AluOpType.mult, op1=AluOpType.max))
            q("sc", nc.scalar.activation(z0[:], a0[:], Act.Exp,
                                         scale=-2.0 * GAMMA, bias=eps[:]))
            q("sc", nc.scalar.activation(p0[:], z0[:], Act.Ln,
                                         bias=1.0, scale=-1.0))
            q("sc", nc.scalar.activation(q0[:], z0[:], Act.Ln,
                                         bias=1.0, scale=1.0))
            q("ve", nc.vector.scalar_tensor_tensor(
                L0f[:], p0[:], LCLAMP, q0[:],
                op0=AluOpType.max, op1=AluOpType.subtract))
            q("ve", nc.vector.tensor_copy(L0[:], L0f[:]))

            # ---------------- PE: warm-ups (keep the p-state ramped
            # through the DMA window), then S0/HL0 half-sweeps in DMA
            # arrival order: S0[0:4] (ma+L0), HL0[0:4] (wa), S0[4:8]
            # (mb), HL0[4:8] (wb).
            for i in range(N_WARM):
                mm(WRMp, xb[:, 0:128], wtiny[:],
                   start=True, stop=True, skip=True)

            def s0_sweep(kos):
                for ko in kos:
                    lo = (ko % 2) * B
                    for ki in range(KCH):
                        mm(s0_ap(ko // 2, lo, lo + B),
                           (ma if ko < 4 else mb)[:, msl(ko, ki)],
                           L0[:, ki * B:(ki + 1) * B],
                           start=(ki == 0), stop=(ki == KCH - 1))

            def hl0_sweep(kos):
                for ko in kos:
                    lo = (ko % 2) * B
                    for ki in range(KCH):
                        mm(hl0_ap(ko // 2, lo, lo + B),
                           (wa if ko < 4 else wb)[:, msl(ko, ki)],
                           xb[:, ki * B:(ki + 1) * B],
                           start=(ki == 0), stop=(ki == KCH - 1))

            s0_sweep(range(0, 4))
            hl0_sweep(range(0, 4))
            hl0_sweep(range(4, 8))
            s0_sweep(range(4, 8))

            # ---------------- trailing chains, one pair per step, with
            # d0 running one step ahead on the scalar queue
            def bsl(p):
                return slice(2 * p * B, (2 * p + 2) * B)

            def d0_step(p):
                q("sc", nc.scalar.activation(d0[:, bsl(p)], s0_ap(p),
                                             Act.Exp))

            def v_h0(p):
                bs = bsl(p)
                q("ve", nc.vector.scalar_tensor_tensor(
                    h0f[:, bs], hl0_ap(p), 0.0, d0[:, bs],
                    op0=AluOpType.max, op1=AluOpType.mult))
                q("ve", nc.vector.tensor_copy(h0b[:, bs], h0f[:, bs]))

            def sc_l1(p):
                bs = bsl(p)
                q("sc", nc.scalar.activation(z1[:, bs], h0f[:, bs], Act.Exp,
                                             scale=-2.0 * GAMMA, bias=eps[:]))
                q("sc", nc.scalar.activation(p1[:, bs], z1[:, bs], Act.Ln,
                                             bias=1.0, scale=-1.0))
                q("sc", nc.scalar.activation(q1[:, bs], z1[:, bs], Act.Ln,
                                             bias=1.0, scale=1.0))

            def v_l1(p):
                bs = bsl(p)
                q("ve", nc.vector.scalar_tensor_tensor(
                    L1f[:, bs], p1[:, bs], LCLAMP, q1[:, bs],
                    op0=AluOpType.max, op1=AluOpType.subtract))
                q("ve", nc.vector.tensor_copy(L1[:, bs], L1f[:, bs]))

            d0_step(0)
            v_h0(0)
            for p in range(NP2):
                if p + 1 < NP2:
                    d0_step(p + 1)
                    v_h0(p + 1)
                sc_l1(p)
                v_l1(p)

            # ---------------- layer-1 + head matmuls, then finale
            for j in range(KCH):
                lsl = slice(j * 128, (j + 1) * 128)
                bsj = slice(j * B, (j + 1) * B)
                mm(S1p, ma[:, lsl], L1[:, bsj],
                   start=(j == 0), stop=(j == KCH - 1), skip=True)
                mm(HL1p, wa[:, lsl], h0b[:, bsj],
                   start=(j == 0), stop=(j == KCH - 1), skip=True)
            # head0 partial for this core's own chunk (slot 0)
            mm(Yp, hd0[:, :], h0f[:, 0:B], start=True, stop=False, skip=True)
            q("sc", nc.scalar.activation(d1[:], S1p, Act.Exp))
            q("ve", nc.vector.scalar_tensor_tensor(
                h1f[:], HL1p, 0.0, d1[:],
                op0=AluOpType.max, op1=AluOpType.mult))
            mm(Yp, hd1[:, :], h1f[:], start=False, stop=True, skip=True)
            q("ve", nc.vector.tensor_copy(yo[:], Yp))
            dma(nc.sync, "s", y_part[:, :], yo[:])
    nc.compile()
    return nc


def _prep(x, v0, fc0, head0, v1, fc1, head1):
    """Host-side weight preprocessing -> per-core in_maps.

    Per core c, layer-0 output-chunk slot k holds chunk (c+k)%8 and
    layer-1 contraction slot k rotates identically, so one SPMD program
    serves all cores.  m1/w1 occupy slot 0 of the mA/wA packs."""
    m0 = (np.asarray(v0) > 0).astype(np.float32)
    w0 = np.asarray(fc0, np.float32) * m0
    m1 = (np.asarray(v1) > 0).astype(np.float32)
    w1 = np.asarray(fc1, np.float32) * m1
    xT = np.asarray(x, np.float32).T                      # [1024, 64]
    xc = np.ascontiguousarray(
        xT.reshape(KCH, 128, B).transpose(1, 0, 2).reshape(128, KB))
    # [ki, ip, oc, op] blocks of the transposed layer-0 weights
    m0blk = m0.T.reshape(KCH, 128, KCH, 128)
    w0blk = w0.T.reshape(KCH, 128, KCH, 128)
    head0 = np.asarray(head0, np.float32)
    head1 = np.asarray(head1, np.float32)
    in_maps = []
    for c in range(N_CORES):
        perm = [(c + k) % KCH for k in range(KCH)]
        # slot-major k, then ki, then op: [ip, k, ki, op] -> [128, 8192]
        m0c = m0blk[:, :, perm, :].transpose(1, 2, 0, 3).reshape(128, -1)
        w0c = w0blk[:, :, perm, :].transpose(1, 2, 0, 3).reshape(128, -1)
        sl = slice(c * OSH, (c + 1) * OSH)
        m1t = m1[sl].T.reshape(KCH, 128, OSH)             # [ic, ip, o]
        w1t = w1[sl].T.reshape(KCH, 128, OSH)
        m1c = m1t[perm].transpose(1, 0, 2).reshape(128, KCH * OSH)
        w1c = w1t[perm].transpose(1, 0, 2).reshape(128, KCH * OSH)
        in_maps.append({
            "xT": xc,
            "mA": np.ascontiguousarray(
                np.concatenate([m1c, m0c[:, :4 * 1024]], axis=1)).astype(M_NP),
            "mB": np.ascontiguousarray(m0c[:, 4 * 1024:]).astype(M_NP),
            "wA": np.ascontiguousarray(
                np.concatenate([w1c, w0c[:, :4 * 1024]], axis=1)
            ).astype(BF16_NP),
            "wB": np.ascontiguousarray(w0c[:, 4 * 1024:]).astype(BF16_NP),
            "hd0T": np.ascontiguousarray(head0[:, sl].T),
            "hd1T": np.ascontiguousarray(head1[:, sl].T),
        })
    return in_maps


def kernel(x, v0, fc0, head0, v1, fc1, head1):
    nc = _CACHE.get("nc")
    if nc is None:
        nc = _CACHE["nc"] = _build()
    in_maps = _prep(x, v0, fc0, head0, v1, fc1, head1)
    kwargs = {}
    if PROFILE["enable"]:
        kwargs = {"trace": True, **PROFILE["trace_kwargs"]}
    res = run_bass_kernel_spmd(nc, in_maps, core_ids=list(range(N_CORES)),
                               **kwargs)
    if PROFILE["enable"]:
        PROFILE["runs"].append(res)
    y = np.zeros((C, B), np.float32)
    for c in range(N_CORES):
        y += res.results[c]["y_part"]
    return np.ascontiguousarray(y.T).astype(np.float32)
